# revision 36
# baseline (speedup 1.0000x reference)
"""Bass/Tile TRN2 kernel for a 3x3 locally-connected (unshared-weight) layer.

Computation (per batch row b, grid unit h, hw = 256*256):
    y[b,h] = sigmoid( sum_o x[b, nbr_idx[o,h]] * (valid[o,h] ? weights[o,h] : 0) )
    y[b,h] = sigmoid(0) = 0.5 where ~fault_mask[h] (mask applied pre-sigmoid)

Strategy: the neighbor gather is a fixed local stencil (verified on host at
call time).  With x transposed to (hw, batch), the layer is a block-banded
matmul: for output chunks of C=126 units, each dy-band's input window is a
128-row slice of x_t, and the per-chunk weight block is a (128, 128)
tridiagonal-ish matrix.  TensorE accumulates 4 window-blocks per chunk
(3x K=128 dy-bands + 1x K=8 merged edge block) into PSUM; four chunks share
a two-bank PSUM tile, and ScalarE applies sigmoid per quad.  Faulted units
output the constant sigmoid(0)=0.5, filled in on the host during unshard.

v2 (DMA-bound baseline at 78us, 20.1MB/core of HBM traffic):
  - x windows and edge-x ship as fp8 e3m4 (scale 2, matmul rhs mixed with
    bf16 lhsT); output ships as fp16, host casts to f32.  13.4MB/core.
  - edge tensors packed: 4 resident DMAs instead of 44 per-slab issues
    (each dma_start costs ~700ns on the issuing engine's queue).
  - odd-chunk edge blocks live at partitions 32:40 so the per-pair edge
    matmuls land in different PE row groups and run concurrently.
  - DMA issue spread: Vector=xw loads, Sync=wm slabs+residents,
    GpSimd=output stores.

Sharding: hw is split 8 ways (66 chunks of 126 units per core, padded grid of
528 chunks); batch (256) rides along the matmul free dimension.  Every core
runs an identical program; boundary effects are encoded in host-built
zero-padded windows / zero weight blocks.
"""

import numpy as np
import ml_dtypes

BATCH = 256
HW = 65536
N_CONN = 9
C = 126               # output chunk size (so a dy-band window is C+2=128 rows)
NCHUNK_PAD = 528      # padded global chunk count, divisible by 8
NCORES = 8
CPC = NCHUNK_PAD // NCORES   # 66 chunks per core
NWIN = 72                    # window slots per core (locals j .. j+4 used)
PAD = 512                    # zero-row padding on each end of x_t
GRID = NCHUNK_PAD * C        # 66528 padded grid extent
SLAB = 6                     # chunks per weight-slab DMA
NSLAB = CPC // SLAB          # 11
XSCALE = 2.0                 # host premultiply on x (e3m4 headroom), undone
                             # by the sigmoid activation's scale=0.5

_BF16 = ml_dtypes.bfloat16
_E3M4 = ml_dtypes.float8_e3m4
_F16 = np.float16


def _build_blocks(weights, nbr_idx, valid):
    """Scatter effective weights into per-chunk matmul blocks.

    Returns (WM, WE) float32 (weight-block column dim padded 126 -> 128 so
    every lhsT has exactly 128 columns, enabling fast weight load):
      WM: (NCHUNK_PAD, 128, 384)  main blocks, free layout [dy0 | dy+1 | dy-1]
      WE: (NCHUNK_PAD, 8, 128)    merged edge blocks (rows 0:4 dy+1, 4:8 dy-1)

    For chunk J (outputs h in [126J, 126J+126)), the 4 pieces read x_t rows:
      P1 main dy0 : window J   rows [126J-1,    126J+127)
      P2 main dy+1: window J+2 rows [126J+251,  126J+379)
      P3 main dy-1: window J-2 rows [126J-253,  126J-125)
      P4 edge rows 0:4  [126J+379, 126J+383),  rows 4:8 [126J-257, 126J-253)
    Raises ValueError if some valid (o,h) connection is not coverable.
    """
    h = np.arange(HW, dtype=np.int64)
    J = h // C
    p = h % C
    g = nbr_idx.astype(np.int64)
    vm = valid.astype(bool)
    w_eff = np.where(vm, weights.astype(np.float32), 0.0)

    Jb = np.broadcast_to(J, g.shape)
    pb = np.broadcast_to(p, g.shape)

    r1 = g - (C * Jb - 1)
    r2 = g - (C * (Jb + 2) - 1)
    r3 = g - (C * (Jb - 2) - 1)
    r4 = g - (C * Jb + 379)            # edge dy+1 -> rows 0:4
    r5 = g - (C * Jb - 257) + 4        # edge dy-1 -> rows 4:8

    in1 = (r1 >= 0) & (r1 < 128)
    in2 = (r2 >= 0) & (r2 < 128)
    in3 = (r3 >= 0) & (r3 < 128)
    in4 = (r4 >= 0) & (r4 < 4)
    in5 = (r5 >= 4) & (r5 < 8)

    m1 = vm & in1
    m2 = vm & in2 & ~m1
    m3 = vm & in3 & ~m1 & ~m2
    m4 = vm & in4 & ~m1 & ~m2 & ~m3
    m5 = vm & in5 & ~m1 & ~m2 & ~m3 & ~m4
    covered = m1 | m2 | m3 | m4 | m5
    if not np.all(covered | ~vm):
        raise ValueError(
            "nbr_idx is not coverable by the local-stencil kernel "
            f"({np.count_nonzero(vm & ~covered)} uncovered connections)"
        )

    WM = np.zeros((NCHUNK_PAD, 128, 384), dtype=np.float32)
    WE = np.zeros((NCHUNK_PAD, 8, 128), dtype=np.float32)
    for m, r, arr, coff in (
        (m1, r1, WM, 0),
        (m2, r2, WM, 128),
        (m3, r3, WM, 256),
        (m4, r4, WE, 0),
        (m5, r5, WE, 0),
    ):
        np.add.at(arr, (Jb[m], r[m], coff + pb[m]), w_eff[m])
    return WM, WE


def _build_program():
    import concourse.bacc as bacc
    import concourse.mybir as mybir
    from concourse import tile
    from concourse._compat import axon_active

    nc = bacc.Bacc(
        "TRN2",
        target_bir_lowering=False,
        debug=not axon_active(),
        num_devices=NCORES,
    )
    f32 = mybir.dt.float32
    bf16 = mybir.dt.bfloat16
    f16 = mybir.dt.float16
    e3 = mybir.dt.float8e3

    xw_d = nc.dram_tensor("xw", [128, NWIN * 256], e3, kind="ExternalInput")
    # wm is chunk-major contiguous per partition so a DMA can span 2 slabs
    # (9.2KB lines -> ~2x the hardware-dynamic queue's per-line rate cap)
    wm_d = nc.dram_tensor("wm", [128, CPC * 384], bf16, kind="ExternalInput")
    # packed edge-x: per slab 3 even-chunk windows then (separately) odd;
    # even rides partitions 0:8, odd partitions 32:40 (distinct PE row
    # groups -> the two per-pair edge matmuls execute concurrently).
    # packed residents: [even | odd] side by side so each is ONE transfer
    # with 16.9KB lines at the head of the sync queue
    xc_d = nc.dram_tensor("xc", [8, 2 * NSLAB * (SLAB // 2) * 256], e3, kind="ExternalInput")
    we_d = nc.dram_tensor("we", [8, 2 * NSLAB * (SLAB // 2) * 128], bf16, kind="ExternalInput")
    yt_d = nc.dram_tensor("yt", [C, CPC * 256], f16, kind="ExternalOutput")

    with tile.TileContext(nc) as tc:
        with (
            tc.tile_pool(name="xw", bufs=1) as xw_pool,
            tc.tile_pool(name="const", bufs=1) as const_pool,
            tc.tile_pool(name="wm", bufs=NSLAB) as wm_pool,
            tc.tile_pool(name="xc", bufs=1) as xc_pool,
            tc.tile_pool(name="we", bufs=1) as we_pool,
            tc.tile_pool(name="out", bufs=6) as out_pool,
            tc.tile_pool(name="psum", bufs=4, space="PSUM") as psum_pool,
        ):
            # resident x windows.  The first 16 windows ride sync/Q1 as one
            # 4KB-line transfer (issued just after the first weight double,
            # below) so the startup chunk set never waits on Q0's squeezed
            # share; the rest stream as 8-window tiles on gpsimd/Q0.
            xw_sizes = [16] + [8] * 7
            xw_base = [sum(xw_sizes[:i]) for i in range(len(xw_sizes))]
            xw_tiles = [
                xw_pool.tile([128, n * 256], e3, tag=f"xw{s}", name=f"xw{s}")
                for s, n in enumerate(xw_sizes)
            ]
            for s, n in enumerate(xw_sizes):
                if s == 0:
                    continue
                nc.gpsimd.dma_start(
                    out=xw_tiles[s][:, :],
                    in_=xw_d[:, xw_base[s] * 256 : (xw_base[s] + n) * 256],
                )

            # resident packed edge tiles: 2 single-transfer DMAs at the
            # head of the sync queue (land well before the first edge MM).
            NXC = NSLAB * (SLAB // 2) * 256
            NWEC = NSLAB * (SLAB // 2) * 128
            xc_all = xc_pool.tile([8, 2 * NXC], e3, tag="xc")
            we_all = we_pool.tile([8, 2 * NWEC], bf16, tag="we")
            nc.sync.dma_start(out=xc_all[:, :], in_=xc_d[:, :])
            nc.sync.dma_start(out=we_all[:, :], in_=we_d[:, :])
            xc_sb = xc_all[:, 0:NXC]
            xco_sb = xc_all[:, NXC : 2 * NXC]
            we_sb = we_all[:, 0:NWEC]
            weo_sb = we_all[:, NWEC : 2 * NWEC]

            # wm DMAs, all on sync/Q1, fully resident (no buffer recycling).
            # First transfer covers slabs 0+1 in one 9.2KB-line double for
            # startup latency; the rest go as singles whose 4.6KB lines
            # split the per-packet round-robin bandwidth roughly evenly
            # with Q0's 4KB packets (xw + output stores).
            wm_tiles = [None] * NSLAB

            def fetch_wm(s0, nsl):
                t = wm_pool.tile([128, 4608], bf16, tag="wmS")
                for k in range(nsl):
                    wm_tiles[s0 + k] = t[:, k * 2304 : (k + 1) * 2304]
                nc.sync.dma_start(
                    out=t[:, 0 : nsl * 2304],
                    in_=wm_d[:, s0 * 2304 : (s0 + nsl) * 2304],
                )

            fetch_wm(0, 2)
            nc.sync.dma_start(
                out=xw_tiles[0][:, :], in_=xw_d[:, 0 : 16 * 256]
            )
            for s in range(2, NSLAB):
                fetch_wm(s, 1)

            # PE pre-warm: dummy matmuls on zeroed SBUF while the first input
            # DMAs are in flight, so the HAM clock-gate opens (1.2 -> 2.4 GHz)
            # before the real matmul stream begins.
            warm_sb = const_pool.tile([128, 640], bf16, tag="warm")
            nc.vector.memset(warm_sb[:, :], 0.0)
            # The bridge is sized to carry the PE, busy and warm, from the
            # first possible matmul (~8us) to when slabs 0-1 + the first
            # xw tiles have certainly landed (~16us): this part's HAM takes
            # >12us of continuous activity to re-open after any idle gap,
            # so one early famine costs far more than an over-long bridge.
            warm_ps = psum_pool.tile([128, 1024], f32, tag="ps")
            for _ in range(30):
                nc.tensor.matmul(
                    warm_ps[:, 0:512],
                    warm_sb[:, 0:128],
                    warm_sb[:, 128:640],
                    start=True,
                    stop=True,
                )

            def win(w):  # rhs AP for local window index w (full 128 rows)
                ti = 0
                while w >= xw_base[ti] + xw_sizes[ti]:
                    ti += 1
                o = w - xw_base[ti]
                return xw_tiles[ti][:, o * 256 : (o + 1) * 256]

            for s in range(NSLAB):
                wm_sb = wm_tiles[s]

                for q2 in range(SLAB // 2):  # chunk pairs within slab
                    pi = s * (SLAB // 2) + q2  # global pair index
                    if pi % 2 == 0:
                        ps = psum_pool.tile([128, 1024], f32)
                    # start=True only on the pair's first MM: it clears the
                    # has_written bits of this pair's whole PSUM bank; every
                    # later MM (start=False) overwrites fresh cells and
                    # accumulates onto written ones, so MM order is free.
                    # window order j, j+2, j+4 so early pairs only touch
                    # early xw tiles (keeps the startup famine-free);
                    # accumulation order within the group is free.
                    for half in range(2):
                        q = q2 * 2 + half
                        j = s * SLAB + q
                        co = (pi % 2) * 512 + half * 256
                        w0 = q * 384
                        nc.tensor.matmul(
                            ps[:, co : co + 256],
                            wm_sb[:, w0 + 256 : w0 + 384],
                            win(j),
                            start=(half == 0),
                            stop=False,
                            skip_group_check=True,
                        )
                        nc.tensor.matmul(
                            ps[:, co : co + 256],
                            wm_sb[:, w0 : w0 + 128],
                            win(j + 2),
                            start=False,
                            stop=False,
                            skip_group_check=True,
                        )
                        nc.tensor.matmul(
                            ps[:, co : co + 256],
                            wm_sb[:, w0 + 128 : w0 + 256],
                            win(j + 4),
                            start=False,
                            stop=False,
                            skip_group_check=True,
                        )
                    # packed edge MMs: even chunk in array row group 0,
                    # odd chunk in row group 1 -- they execute concurrently.
                    co = (pi % 2) * 512
                    nc.tensor.matmul(
                        ps[:, co : co + 256],
                        we_sb[0:8, pi * 128 : (pi + 1) * 128],
                        xc_sb[0:8, pi * 256 : (pi + 1) * 256],
                        start=False,
                        stop=False,
                        skip_group_check=True,
                    )
                    nc.tensor.matmul(
                        ps[:, co + 256 : co + 512],
                        weo_sb[0:8, pi * 128 : (pi + 1) * 128],
                        xco_sb[0:8, pi * 256 : (pi + 1) * 256],
                        start=False,
                        stop=True,
                        skip_group_check=True,
                    )

                    npair = NSLAB * (SLAB // 2)
                    if s == NSLAB - 1:
                        # tail: per-pair sigmoid+store so the final store is
                        # small and starts right after the last matmuls
                        ot = out_pool.tile([128, 1024], f16)
                        co = (pi % 2) * 512
                        nc.scalar.activation(
                            ot[0:C, 0:512],
                            ps[0:C, co : co + 512],
                            mybir.ActivationFunctionType.Sigmoid,
                            bias=0.0,
                            scale=1.0 / XSCALE,
                        )
                        nc.gpsimd.dma_start(
                            out=yt_d[:, pi * 512 : (pi + 1) * 512],
                            in_=ot[0:C, 0:512],
                        )
                    elif pi % 2 == 1 or pi == npair - 1:
                        width = (pi % 2 + 1) * 512
                        ot = out_pool.tile([128, 1024], f16)
                        nc.scalar.activation(
                            ot[0:C, 0:width],
                            ps[0:C, 0:width],
                            mybir.ActivationFunctionType.Sigmoid,
                            bias=0.0,
                            scale=1.0 / XSCALE,
                        )
                        j0 = (pi // 2) * 4  # first chunk of this store group
                        nc.gpsimd.dma_start(
                            out=yt_d[:, j0 * 256 : j0 * 256 + width],
                            in_=ot[0:C, 0:width],
                        )
    nc.compile()
    return nc


TRACE = False          # set by test harness to capture an NTFF profile
LAST_RESULTS = None    # BassKernelResults of the most recent run
_NC_CACHE = None       # compiled program, reused across calls


def kernel(x, weights, nbr_idx, valid, fault_mask):
    global LAST_RESULTS
    from concourse.bass_utils import run_bass_kernel_spmd

    x = np.asarray(x)
    out_dtype = x.dtype

    WM, WE = _build_blocks(np.asarray(weights), np.asarray(nbr_idx), np.asarray(valid))
    WM = WM.astype(_BF16)
    WE = WE.astype(_BF16)

    # x transposed to (hw, batch), zero-padded, scaled into e3m4 headroom
    xt_pad = np.zeros((PAD + GRID + PAD, BATCH), dtype=_E3M4)
    xt_pad[PAD : PAD + HW] = np.clip(
        np.ascontiguousarray(x.T) * np.float32(XSCALE), -15.5, 15.5
    ).astype(_E3M4)

    k128 = np.arange(128)
    in_maps = []
    for c in range(NCORES):
        j0 = c * CPC
        # main windows: global window t in [j0-2, j0+70), rows PAD + 126*t - 1 + k
        tglob = j0 - 2 + np.arange(NWIN)
        rows = (PAD + C * tglob - 1)[:, None] + k128[None, :]  # (72, 128)
        xw = np.ascontiguousarray(xt_pad[rows].transpose(1, 0, 2))  # (128, 72, 256)

        # merged edge windows: rows 0:4 from [126J+379, +4), 4:8 from [126J-257, +4)
        Jc = j0 + np.arange(CPC)
        k4 = np.arange(4)
        erows_p = (PAD + C * Jc + 379)[:, None] + k4[None, :]  # (66, 4)
        erows_m = (PAD + C * Jc - 257)[:, None] + k4[None, :]  # (66, 4)
        xc = np.concatenate(
            [xt_pad[erows_p], xt_pad[erows_m]], axis=1
        )  # (66, 8, 256)
        xce = xc[0::2].transpose(1, 0, 2)  # (8, 33, 256)
        xco = xc[1::2].transpose(1, 0, 2)

        wm_c = WM[j0 : j0 + CPC].transpose(1, 0, 2)  # (128, 66, 384)
        we_cc = WE[j0 : j0 + CPC]
        wee = we_cc[0::2].transpose(1, 0, 2)  # (8, 33, 128)
        weo = we_cc[1::2].transpose(1, 0, 2)

        in_maps.append(
            {
                "xw": xw.reshape(128, NWIN * 256),
                "wm": np.ascontiguousarray(wm_c).reshape(128, CPC * 384),
                "xc": np.ascontiguousarray(
                    np.concatenate(
                        [xce.reshape(8, -1), xco.reshape(8, -1)], axis=1
                    )
                ),
                "we": np.ascontiguousarray(
                    np.concatenate(
                        [wee.reshape(8, -1), weo.reshape(8, -1)], axis=1
                    )
                ),
            }
        )

    global _NC_CACHE
    if _NC_CACHE is None:
        _NC_CACHE = _build_program()
    nc = _NC_CACHE
    res = run_bass_kernel_spmd(
        nc, in_maps, core_ids=list(range(NCORES)), trace=TRACE
    )
    LAST_RESULTS = res

    # unshard: per-core yt is (126, 66*256) partition-major -> (B, HW)
    yts = [
        r["yt"].reshape(C, CPC, BATCH).transpose(1, 0, 2).reshape(CPC * C, BATCH)
        for r in res.results
    ]
    yt = np.concatenate(yts, axis=0)  # (66528, 256)
    y = np.ascontiguousarray(yt[:HW].T).astype(np.float32, copy=False)
    # faulted units: reference computes sigmoid(where(fault, y, 0)) -> 0.5
    fault = np.asarray(fault_mask).astype(bool)
    y[:, ~fault] = np.float32(0.5)
    return y.astype(out_dtype, copy=False)


# revision 43
# speedup vs baseline: 1.0198x; 1.0198x over previous
"""Bass/Tile TRN2 kernel for a 3x3 locally-connected (unshared-weight) layer.

Computation (per batch row b, grid unit h, hw = 256*256):
    y[b,h] = sigmoid( sum_o x[b, nbr_idx[o,h]] * (valid[o,h] ? weights[o,h] : 0) )
    y[b,h] = sigmoid(0) = 0.5 where ~fault_mask[h] (mask applied pre-sigmoid)

Strategy: the neighbor gather is a fixed local stencil (verified on host at
call time).  With x transposed to (hw, batch), the layer is a block-banded
matmul: for output chunks of C=126 units, each dy-band's input window is a
128-row slice of x_t, and the per-chunk weight block is a (128, 128)
tridiagonal-ish matrix.  TensorE accumulates 4 window-blocks per chunk
(3x K=128 dy-bands + 1x K=8 merged edge block) into PSUM; four chunks share
a two-bank PSUM tile, and ScalarE applies sigmoid per quad.  Faulted units
output the constant sigmoid(0)=0.5, filled in on the host during unshard.

v2 (DMA-bound baseline at 78us, 20.1MB/core of HBM traffic):
  - x windows and edge-x ship as fp8 e3m4 (scale 2, matmul rhs mixed with
    bf16 lhsT); output ships as fp16, host casts to f32.  13.4MB/core.
  - edge tensors packed: 4 resident DMAs instead of 44 per-slab issues
    (each dma_start costs ~700ns on the issuing engine's queue).
  - odd-chunk edge blocks live at partitions 32:40 so the per-pair edge
    matmuls land in different PE row groups and run concurrently.
  - DMA issue spread: Vector=xw loads, Sync=wm slabs+residents,
    GpSimd=output stores.

Sharding: hw is split 8 ways (66 chunks of 126 units per core, padded grid of
528 chunks); batch (256) rides along the matmul free dimension.  Every core
runs an identical program; boundary effects are encoded in host-built
zero-padded windows / zero weight blocks.
"""

import numpy as np
import ml_dtypes

BATCH = 256
HW = 65536
N_CONN = 9
C = 126               # output chunk size (so a dy-band window is C+2=128 rows)
NCHUNK_PAD = 528      # padded global chunk count, divisible by 8
NCORES = 8
CPC = NCHUNK_PAD // NCORES   # 66 chunks per core
NWIN = 72                    # window slots per core (locals j .. j+4 used)
PAD = 512                    # zero-row padding on each end of x_t
GRID = NCHUNK_PAD * C        # 66528 padded grid extent
SLAB = 6                     # chunks per weight-slab DMA
NSLAB = CPC // SLAB          # 11
XSCALE = 2.0                 # host premultiply on x (e3m4 headroom), undone
                             # by the sigmoid activation's scale=0.5

_BF16 = ml_dtypes.bfloat16
_E3M4 = ml_dtypes.float8_e3m4
_F16 = np.float16


def _build_blocks(weights, nbr_idx, valid):
    """Scatter effective weights into per-chunk matmul blocks.

    Returns (WM, WE) float32 (weight-block column dim padded 126 -> 128 so
    every lhsT has exactly 128 columns, enabling fast weight load):
      WM: (NCHUNK_PAD, 128, 384)  main blocks, free layout [dy0 | dy+1 | dy-1]
      WE: (NCHUNK_PAD, 8, 128)    merged edge blocks (rows 0:4 dy+1, 4:8 dy-1)

    For chunk J (outputs h in [126J, 126J+126)), the 4 pieces read x_t rows:
      P1 main dy0 : window J   rows [126J-1,    126J+127)
      P2 main dy+1: window J+2 rows [126J+251,  126J+379)
      P3 main dy-1: window J-2 rows [126J-253,  126J-125)
      P4 edge rows 0:4  [126J+379, 126J+383),  rows 4:8 [126J-257, 126J-253)
    Raises ValueError if some valid (o,h) connection is not coverable.
    """
    h = np.arange(HW, dtype=np.int64)
    J = h // C
    p = h % C
    g = nbr_idx.astype(np.int64)
    vm = valid.astype(bool)
    w_eff = np.where(vm, weights.astype(np.float32), 0.0)

    Jb = np.broadcast_to(J, g.shape)
    pb = np.broadcast_to(p, g.shape)

    r1 = g - (C * Jb - 1)
    r2 = g - (C * (Jb + 2) - 1)
    r3 = g - (C * (Jb - 2) - 1)
    r4 = g - (C * Jb + 379)            # edge dy+1 -> rows 0:4
    r5 = g - (C * Jb - 257) + 4        # edge dy-1 -> rows 4:8

    in1 = (r1 >= 0) & (r1 < 128)
    in2 = (r2 >= 0) & (r2 < 128)
    in3 = (r3 >= 0) & (r3 < 128)
    in4 = (r4 >= 0) & (r4 < 4)
    in5 = (r5 >= 4) & (r5 < 8)

    m1 = vm & in1
    m2 = vm & in2 & ~m1
    m3 = vm & in3 & ~m1 & ~m2
    m4 = vm & in4 & ~m1 & ~m2 & ~m3
    m5 = vm & in5 & ~m1 & ~m2 & ~m3 & ~m4
    covered = m1 | m2 | m3 | m4 | m5
    if not np.all(covered | ~vm):
        raise ValueError(
            "nbr_idx is not coverable by the local-stencil kernel "
            f"({np.count_nonzero(vm & ~covered)} uncovered connections)"
        )

    WM = np.zeros((NCHUNK_PAD, 128, 384), dtype=np.float32)
    WE = np.zeros((NCHUNK_PAD, 8, 128), dtype=np.float32)
    for m, r, arr, coff in (
        (m1, r1, WM, 0),
        (m2, r2, WM, 128),
        (m3, r3, WM, 256),
        (m4, r4, WE, 0),
        (m5, r5, WE, 0),
    ):
        np.add.at(arr, (Jb[m], r[m], coff + pb[m]), w_eff[m])
    return WM, WE


def _build_program():
    import concourse.bacc as bacc
    import concourse.mybir as mybir
    from concourse import tile
    from concourse._compat import axon_active

    nc = bacc.Bacc(
        "TRN2",
        target_bir_lowering=False,
        debug=not axon_active(),
        num_devices=NCORES,
    )
    f32 = mybir.dt.float32
    bf16 = mybir.dt.bfloat16
    f16 = mybir.dt.float16
    e3 = mybir.dt.float8e3

    xw_d = nc.dram_tensor("xw", [128, NWIN * 256], e3, kind="ExternalInput")
    # wm is chunk-major contiguous per partition so a DMA can span 2 slabs
    # (9.2KB lines -> ~2x the hardware-dynamic queue's per-line rate cap)
    wm_d = nc.dram_tensor("wm", [128, CPC * 384], bf16, kind="ExternalInput")
    # packed edge-x: per slab 3 even-chunk windows then (separately) odd;
    # even rides partitions 0:8, odd partitions 32:40 (distinct PE row
    # groups -> the two per-pair edge matmuls execute concurrently).
    # packed residents: [even | odd] side by side so each is ONE transfer
    # with 16.9KB lines at the head of the sync queue
    xc_d = nc.dram_tensor("xc", [8, 2 * NSLAB * (SLAB // 2) * 256], e3, kind="ExternalInput")
    we_d = nc.dram_tensor("we", [8, 2 * NSLAB * (SLAB // 2) * 128], bf16, kind="ExternalInput")
    yt_d = nc.dram_tensor("yt", [C, CPC * 256], f16, kind="ExternalOutput")

    with tile.TileContext(nc) as tc:
        with (
            tc.tile_pool(name="xw", bufs=1) as xw_pool,
            tc.tile_pool(name="const", bufs=1) as const_pool,
            tc.tile_pool(name="wm", bufs=NSLAB - 1) as wm_pool,
            tc.tile_pool(name="wmp", bufs=SLAB // 2) as wmp_pool,
            tc.tile_pool(name="xc", bufs=1) as xc_pool,
            tc.tile_pool(name="we", bufs=1) as we_pool,
            tc.tile_pool(name="out", bufs=6) as out_pool,
            tc.tile_pool(name="psum", bufs=4, space="PSUM") as psum_pool,
        ):
            # resident x windows, 9 tiles of 8 windows.  Tiles 0-1 ride
            # sync/Q1 interleaved with the startup weights (below); the
            # rest stream on gpsimd/Q0.
            xw_sizes = [8] * 9
            xw_base = [sum(xw_sizes[:i]) for i in range(len(xw_sizes))]
            xw_tiles = [
                xw_pool.tile([128, n * 256], e3, tag=f"xw{s}", name=f"xw{s}")
                for s, n in enumerate(xw_sizes)
            ]
            for s, n in enumerate(xw_sizes):
                if s < 2:
                    continue
                nc.gpsimd.dma_start(
                    out=xw_tiles[s][:, :],
                    in_=xw_d[:, xw_base[s] * 256 : (xw_base[s] + n) * 256],
                )

            # resident packed edge tiles: 2 single-transfer DMAs at the
            # head of the sync queue (land well before the first edge MM).
            NXC = NSLAB * (SLAB // 2) * 256
            NWEC = NSLAB * (SLAB // 2) * 128
            xc_all = xc_pool.tile([8, 2 * NXC], e3, tag="xc")
            we_all = we_pool.tile([8, 2 * NWEC], bf16, tag="we")
            xc_sb = xc_all[:, 0:NXC]
            xco_sb = xc_all[:, NXC : 2 * NXC]
            we_sb = we_all[:, 0:NWEC]
            weo_sb = we_all[:, NWEC : 2 * NWEC]

            # Startup stream on sync/Q1 in exact consumption order: slab 0's
            # weights go PAIR-granular (196KB lumps) interleaved with the
            # first two xw tiles.  The DMA fabric ramps slowly for its first
            # ~6us, so coarse startup lumps produce one long matmul famine
            # (which trips the HAM clock-gate: >12us at half clock); fine
            # lumps produce harmless sub-us stall-and-go instead.  Slabs
            # 1..10 follow as 4.6KB-line singles whose packets split the
            # round-robin bandwidth roughly evenly with Q0.
            wmp_tiles = []
            for p3 in range(SLAB // 2):
                t = wmp_pool.tile([128, 768], bf16, tag="wmP")
                wmp_tiles.append(t)
            wm_tiles = [None] * NSLAB

            def fetch_wm_pair(p3):
                nc.sync.dma_start(
                    out=wmp_tiles[p3][:, :],
                    in_=wm_d[:, p3 * 768 : (p3 + 1) * 768],
                )

            def fetch_wm(s):
                t = wm_pool.tile([128, 2304], bf16, tag="wmS")
                wm_tiles[s] = t
                nc.sync.dma_start(
                    out=t[:, :], in_=wm_d[:, s * 2304 : (s + 1) * 2304]
                )

            fetch_wm_pair(0)
            nc.sync.dma_start(out=xw_tiles[0][:, :], in_=xw_d[:, 0 : 8 * 256])
            nc.sync.dma_start(out=xc_all[:, :], in_=xc_d[:, :])
            nc.sync.dma_start(out=we_all[:, :], in_=we_d[:, :])
            fetch_wm_pair(1)
            fetch_wm_pair(2)
            nc.sync.dma_start(
                out=xw_tiles[1][:, :], in_=xw_d[:, 8 * 256 : 16 * 256]
            )
            for s in range(1, NSLAB):
                fetch_wm(s)

            # PE pre-warm: dummy matmuls on zeroed SBUF while the first input
            # DMAs are in flight, so the HAM clock-gate opens (1.2 -> 2.4 GHz)
            # before the real matmul stream begins.
            warm_sb = const_pool.tile([128, 640], bf16, tag="warm")
            nc.vector.memset(warm_sb[:, :], 0.0)
            # The bridge is sized to carry the PE, busy and warm, from the
            # first possible matmul (~8us) to when slabs 0-1 + the first
            # xw tiles have certainly landed (~16us): this part's HAM takes
            # >12us of continuous activity to re-open after any idle gap,
            # so one early famine costs far more than an over-long bridge.
            warm_ps = psum_pool.tile([128, 1024], f32, tag="ps")
            for _ in range(30):
                nc.tensor.matmul(
                    warm_ps[:, 0:512],
                    warm_sb[:, 0:128],
                    warm_sb[:, 128:640],
                    start=True,
                    stop=True,
                )

            def win(w):  # rhs AP for local window index w (full 128 rows)
                ti = 0
                while w >= xw_base[ti] + xw_sizes[ti]:
                    ti += 1
                o = w - xw_base[ti]
                return xw_tiles[ti][:, o * 256 : (o + 1) * 256]

            for s in range(NSLAB):
                for q2 in range(SLAB // 2):  # chunk pairs within slab
                    # slab 0 weights live in pair-granular startup tiles
                    if s == 0:
                        wm_sb = wmp_tiles[q2][:, :]
                        wq0 = -q2 * 768  # so q*384 indexes within the pair
                    else:
                        wm_sb = wm_tiles[s][:, :]
                        wq0 = 0
                    pi = s * (SLAB // 2) + q2  # global pair index
                    if pi % 2 == 0:
                        ps = psum_pool.tile([128, 1024], f32)
                    # start=True only on the pair's first MM: it clears the
                    # has_written bits of this pair's whole PSUM bank; every
                    # later MM (start=False) overwrites fresh cells and
                    # accumulates onto written ones, so MM order is free.
                    # window order j, j+2, j+4 so early pairs only touch
                    # early xw tiles (keeps the startup famine-free);
                    # accumulation order within the group is free.
                    for half in range(2):
                        q = q2 * 2 + half
                        j = s * SLAB + q
                        co = (pi % 2) * 512 + half * 256
                        w0 = wq0 + q * 384
                        nc.tensor.matmul(
                            ps[:, co : co + 256],
                            wm_sb[:, w0 + 256 : w0 + 384],
                            win(j),
                            start=(half == 0),
                            stop=False,
                            skip_group_check=True,
                        )
                        nc.tensor.matmul(
                            ps[:, co : co + 256],
                            wm_sb[:, w0 : w0 + 128],
                            win(j + 2),
                            start=False,
                            stop=False,
                            skip_group_check=True,
                        )
                        nc.tensor.matmul(
                            ps[:, co : co + 256],
                            wm_sb[:, w0 + 128 : w0 + 256],
                            win(j + 4),
                            start=False,
                            stop=False,
                            skip_group_check=True,
                        )
                    # packed edge MMs: even chunk in array row group 0,
                    # odd chunk in row group 1 -- they execute concurrently.
                    co = (pi % 2) * 512
                    nc.tensor.matmul(
                        ps[:, co : co + 256],
                        we_sb[0:8, pi * 128 : (pi + 1) * 128],
                        xc_sb[0:8, pi * 256 : (pi + 1) * 256],
                        start=False,
                        stop=False,
                        skip_group_check=True,
                    )
                    nc.tensor.matmul(
                        ps[:, co + 256 : co + 512],
                        weo_sb[0:8, pi * 128 : (pi + 1) * 128],
                        xco_sb[0:8, pi * 256 : (pi + 1) * 256],
                        start=False,
                        stop=True,
                        skip_group_check=True,
                    )

                    npair = NSLAB * (SLAB // 2)
                    if s == NSLAB - 1:
                        # tail: per-pair sigmoid+store so the final store is
                        # small and starts right after the last matmuls
                        ot = out_pool.tile([128, 1024], f16)
                        co = (pi % 2) * 512
                        nc.scalar.activation(
                            ot[0:C, 0:512],
                            ps[0:C, co : co + 512],
                            mybir.ActivationFunctionType.Sigmoid,
                            bias=0.0,
                            scale=1.0 / XSCALE,
                        )
                        nc.gpsimd.dma_start(
                            out=yt_d[:, pi * 512 : (pi + 1) * 512],
                            in_=ot[0:C, 0:512],
                        )
                    elif pi % 2 == 1 or pi == npair - 1:
                        width = (pi % 2 + 1) * 512
                        ot = out_pool.tile([128, 1024], f16)
                        nc.scalar.activation(
                            ot[0:C, 0:width],
                            ps[0:C, 0:width],
                            mybir.ActivationFunctionType.Sigmoid,
                            bias=0.0,
                            scale=1.0 / XSCALE,
                        )
                        j0 = (pi // 2) * 4  # first chunk of this store group
                        nc.gpsimd.dma_start(
                            out=yt_d[:, j0 * 256 : j0 * 256 + width],
                            in_=ot[0:C, 0:width],
                        )
    nc.compile()
    return nc


TRACE = False          # set by test harness to capture an NTFF profile
LAST_RESULTS = None    # BassKernelResults of the most recent run
_NC_CACHE = None       # compiled program, reused across calls


def kernel(x, weights, nbr_idx, valid, fault_mask):
    global LAST_RESULTS
    from concourse.bass_utils import run_bass_kernel_spmd

    x = np.asarray(x)
    out_dtype = x.dtype

    WM, WE = _build_blocks(np.asarray(weights), np.asarray(nbr_idx), np.asarray(valid))
    WM = WM.astype(_BF16)
    WE = WE.astype(_BF16)

    # x transposed to (hw, batch), zero-padded, scaled into e3m4 headroom
    xt_pad = np.zeros((PAD + GRID + PAD, BATCH), dtype=_E3M4)
    xt_pad[PAD : PAD + HW] = np.clip(
        np.ascontiguousarray(x.T) * np.float32(XSCALE), -15.5, 15.5
    ).astype(_E3M4)

    k128 = np.arange(128)
    in_maps = []
    for c in range(NCORES):
        j0 = c * CPC
        # main windows: global window t in [j0-2, j0+70), rows PAD + 126*t - 1 + k
        tglob = j0 - 2 + np.arange(NWIN)
        rows = (PAD + C * tglob - 1)[:, None] + k128[None, :]  # (72, 128)
        xw = np.ascontiguousarray(xt_pad[rows].transpose(1, 0, 2))  # (128, 72, 256)

        # merged edge windows: rows 0:4 from [126J+379, +4), 4:8 from [126J-257, +4)
        Jc = j0 + np.arange(CPC)
        k4 = np.arange(4)
        erows_p = (PAD + C * Jc + 379)[:, None] + k4[None, :]  # (66, 4)
        erows_m = (PAD + C * Jc - 257)[:, None] + k4[None, :]  # (66, 4)
        xc = np.concatenate(
            [xt_pad[erows_p], xt_pad[erows_m]], axis=1
        )  # (66, 8, 256)
        xce = xc[0::2].transpose(1, 0, 2)  # (8, 33, 256)
        xco = xc[1::2].transpose(1, 0, 2)

        wm_c = WM[j0 : j0 + CPC].transpose(1, 0, 2)  # (128, 66, 384)
        we_cc = WE[j0 : j0 + CPC]
        wee = we_cc[0::2].transpose(1, 0, 2)  # (8, 33, 128)
        weo = we_cc[1::2].transpose(1, 0, 2)

        in_maps.append(
            {
                "xw": xw.reshape(128, NWIN * 256),
                "wm": np.ascontiguousarray(wm_c).reshape(128, CPC * 384),
                "xc": np.ascontiguousarray(
                    np.concatenate(
                        [xce.reshape(8, -1), xco.reshape(8, -1)], axis=1
                    )
                ),
                "we": np.ascontiguousarray(
                    np.concatenate(
                        [wee.reshape(8, -1), weo.reshape(8, -1)], axis=1
                    )
                ),
            }
        )

    global _NC_CACHE
    if _NC_CACHE is None:
        _NC_CACHE = _build_program()
    nc = _NC_CACHE
    res = run_bass_kernel_spmd(
        nc, in_maps, core_ids=list(range(NCORES)), trace=TRACE
    )
    LAST_RESULTS = res

    # unshard: per-core yt is (126, 66*256) partition-major -> (B, HW)
    yts = [
        r["yt"].reshape(C, CPC, BATCH).transpose(1, 0, 2).reshape(CPC * C, BATCH)
        for r in res.results
    ]
    yt = np.concatenate(yts, axis=0)  # (66528, 256)
    y = np.ascontiguousarray(yt[:HW].T).astype(np.float32, copy=False)
    # faulted units: reference computes sigmoid(where(fault, y, 0)) -> 0.5
    fault = np.asarray(fault_mask).astype(bool)
    y[:, ~fault] = np.float32(0.5)
    return y.astype(out_dtype, copy=False)


# revision 45
# speedup vs baseline: 1.1506x; 1.1283x over previous
"""Bass/Tile TRN2 kernel for a 3x3 locally-connected (unshared-weight) layer.

Computation (per batch row b, grid unit h, hw = 256*256):
    y[b,h] = sigmoid( sum_o x[b, nbr_idx[o,h]] * (valid[o,h] ? weights[o,h] : 0) )
    y[b,h] = sigmoid(0) = 0.5 where ~fault_mask[h] (mask applied pre-sigmoid)

Strategy: the neighbor gather is a fixed local stencil (verified on host at
call time).  With x transposed to (hw, batch), the layer is a block-banded
matmul: for output chunks of C=126 units, each dy-band's input window is a
128-row slice of x_t, and the per-chunk weight block is a (128, 128)
tridiagonal-ish matrix.  TensorE accumulates 4 window-blocks per chunk
(3x K=128 dy-bands + 1x K=8 merged edge block) into PSUM; four chunks share
a two-bank PSUM tile, and ScalarE applies sigmoid per quad.  Faulted units
output the constant sigmoid(0)=0.5, filled in on the host during unshard.

v2 (DMA-bound baseline at 78us, 20.1MB/core of HBM traffic):
  - x windows and edge-x ship as fp8 e3m4 (scale 2, matmul rhs mixed with
    bf16 lhsT); output ships as fp16, host casts to f32.  13.4MB/core.
  - edge tensors packed: 4 resident DMAs instead of 44 per-slab issues
    (each dma_start costs ~700ns on the issuing engine's queue).
  - odd-chunk edge blocks live at partitions 32:40 so the per-pair edge
    matmuls land in different PE row groups and run concurrently.
  - DMA issue spread: Vector=xw loads, Sync=wm slabs+residents,
    GpSimd=output stores.

Sharding: hw is split 8 ways (66 chunks of 126 units per core, padded grid of
528 chunks); batch (256) rides along the matmul free dimension.  Every core
runs an identical program; boundary effects are encoded in host-built
zero-padded windows / zero weight blocks.
"""

import numpy as np
import ml_dtypes

BATCH = 256
HW = 65536
N_CONN = 9
C = 126               # output chunk size (so a dy-band window is C+2=128 rows)
NCHUNK_PAD = 528      # padded global chunk count, divisible by 8
NCORES = 8
CPC = NCHUNK_PAD // NCORES   # 66 chunks per core
NWIN = 72                    # window slots per core (locals j .. j+4 used)
PAD = 512                    # zero-row padding on each end of x_t
GRID = NCHUNK_PAD * C        # 66528 padded grid extent
SLAB = 6                     # chunks per weight-slab DMA
NSLAB = CPC // SLAB          # 11
XSCALE = 2.0                 # host premultiply on x (e3m4 headroom), undone
                             # by the sigmoid activation's scale=0.5

_BF16 = ml_dtypes.bfloat16
_E3M4 = ml_dtypes.float8_e3m4
_F16 = np.float16


def _build_blocks(weights, nbr_idx, valid):
    """Scatter effective weights into per-chunk matmul blocks.

    Returns (WM, WE) float32 (weight-block column dim padded 126 -> 128 so
    every lhsT has exactly 128 columns, enabling fast weight load):
      WM: (NCHUNK_PAD, 128, 384)  main blocks, free layout [dy0 | dy+1 | dy-1]
      WE: (NCHUNK_PAD, 8, 128)    merged edge blocks (rows 0:4 dy+1, 4:8 dy-1)

    For chunk J (outputs h in [126J, 126J+126)), the 4 pieces read x_t rows:
      P1 main dy0 : window J   rows [126J-1,    126J+127)
      P2 main dy+1: window J+2 rows [126J+251,  126J+379)
      P3 main dy-1: window J-2 rows [126J-253,  126J-125)
      P4 edge rows 0:4  [126J+379, 126J+383),  rows 4:8 [126J-257, 126J-253)
    Raises ValueError if some valid (o,h) connection is not coverable.
    """
    h = np.arange(HW, dtype=np.int64)
    J = h // C
    p = h % C
    g = nbr_idx.astype(np.int64)
    vm = valid.astype(bool)
    w_eff = np.where(vm, weights.astype(np.float32), 0.0)

    Jb = np.broadcast_to(J, g.shape)
    pb = np.broadcast_to(p, g.shape)

    r1 = g - (C * Jb - 1)
    r2 = g - (C * (Jb + 2) - 1)
    r3 = g - (C * (Jb - 2) - 1)
    r4 = g - (C * Jb + 379)            # edge dy+1 -> rows 0:4
    r5 = g - (C * Jb - 257) + 4        # edge dy-1 -> rows 4:8

    in1 = (r1 >= 0) & (r1 < 128)
    in2 = (r2 >= 0) & (r2 < 128)
    in3 = (r3 >= 0) & (r3 < 128)
    in4 = (r4 >= 0) & (r4 < 4)
    in5 = (r5 >= 4) & (r5 < 8)

    m1 = vm & in1
    m2 = vm & in2 & ~m1
    m3 = vm & in3 & ~m1 & ~m2
    m4 = vm & in4 & ~m1 & ~m2 & ~m3
    m5 = vm & in5 & ~m1 & ~m2 & ~m3 & ~m4
    covered = m1 | m2 | m3 | m4 | m5
    if not np.all(covered | ~vm):
        raise ValueError(
            "nbr_idx is not coverable by the local-stencil kernel "
            f"({np.count_nonzero(vm & ~covered)} uncovered connections)"
        )

    WM = np.zeros((NCHUNK_PAD, 128, 384), dtype=np.float32)
    WE = np.zeros((NCHUNK_PAD, 8, 128), dtype=np.float32)
    for m, r, arr, coff in (
        (m1, r1, WM, 0),
        (m2, r2, WM, 128),
        (m3, r3, WM, 256),
        (m4, r4, WE, 0),
        (m5, r5, WE, 0),
    ):
        np.add.at(arr, (Jb[m], r[m], coff + pb[m]), w_eff[m])
    return WM, WE


def _build_program():
    import concourse.bacc as bacc
    import concourse.mybir as mybir
    from concourse import tile
    from concourse._compat import axon_active

    nc = bacc.Bacc(
        "TRN2",
        target_bir_lowering=False,
        debug=not axon_active(),
        num_devices=NCORES,
    )
    f32 = mybir.dt.float32
    bf16 = mybir.dt.bfloat16
    f16 = mybir.dt.float16
    e3 = mybir.dt.float8e3

    xw_d = nc.dram_tensor("xw", [128, NWIN * 256], e3, kind="ExternalInput")
    # wm is chunk-major contiguous per partition so a DMA can span 2 slabs
    # (9.2KB lines -> ~2x the hardware-dynamic queue's per-line rate cap)
    wm_d = nc.dram_tensor("wm", [128, CPC * 384], bf16, kind="ExternalInput")
    # packed edge-x: per slab 3 even-chunk windows then (separately) odd;
    # even rides partitions 0:8, odd partitions 32:40 (distinct PE row
    # groups -> the two per-pair edge matmuls execute concurrently).
    # packed residents: [even | odd] side by side so each is ONE transfer
    # with 16.9KB lines at the head of the sync queue
    xc_d = nc.dram_tensor("xc", [8, 2 * NSLAB * (SLAB // 2) * 256], e3, kind="ExternalInput")
    we_d = nc.dram_tensor("we", [8, 2 * NSLAB * (SLAB // 2) * 128], bf16, kind="ExternalInput")
    yt_d = nc.dram_tensor("yt", [C, CPC * 256], f16, kind="ExternalOutput")

    with tile.TileContext(nc) as tc:
        with (
            tc.tile_pool(name="xw", bufs=1) as xw_pool,
            tc.tile_pool(name="const", bufs=1) as const_pool,
            tc.tile_pool(name="wm", bufs=NSLAB - 1) as wm_pool,
            tc.tile_pool(name="wmp", bufs=SLAB // 2) as wmp_pool,
            tc.tile_pool(name="xc", bufs=1) as xc_pool,
            tc.tile_pool(name="we", bufs=1) as we_pool,
            tc.tile_pool(name="out", bufs=6) as out_pool,
            tc.tile_pool(name="psum", bufs=4, space="PSUM") as psum_pool,
        ):
            # resident x windows, 9 tiles of 8 windows.  Tiles 0-1 ride
            # sync/Q1 interleaved with the startup weights (below); the
            # rest stream on gpsimd/Q0.
            xw_sizes = [8] * 9
            xw_base = [sum(xw_sizes[:i]) for i in range(len(xw_sizes))]
            xw_tiles = [
                xw_pool.tile([128, n * 256], e3, tag=f"xw{s}", name=f"xw{s}")
                for s, n in enumerate(xw_sizes)
            ]
            for s, n in enumerate(xw_sizes):
                if s < 2:
                    continue
                nc.gpsimd.dma_start(
                    out=xw_tiles[s][:, :],
                    in_=xw_d[:, xw_base[s] * 256 : (xw_base[s] + n) * 256],
                )

            # resident packed edge tiles: 2 single-transfer DMAs at the
            # head of the sync queue (land well before the first edge MM).
            NXC = NSLAB * (SLAB // 2) * 256
            NWEC = NSLAB * (SLAB // 2) * 128
            xc_all = xc_pool.tile([8, 2 * NXC], e3, tag="xc")
            we_all = we_pool.tile([8, 2 * NWEC], bf16, tag="we")
            xc_sb = xc_all[:, 0:NXC]
            xco_sb = xc_all[:, NXC : 2 * NXC]
            we_sb = we_all[:, 0:NWEC]
            weo_sb = we_all[:, NWEC : 2 * NWEC]

            # Startup stream on sync/Q1 in exact consumption order: slab 0's
            # weights go PAIR-granular (196KB lumps) interleaved with the
            # first two xw tiles.  The DMA fabric ramps slowly for its first
            # ~6us, so coarse startup lumps produce one long matmul famine
            # (which trips the HAM clock-gate: >12us at half clock); fine
            # lumps produce harmless sub-us stall-and-go instead.  Slabs
            # 1..10 follow as 4.6KB-line singles whose packets split the
            # round-robin bandwidth roughly evenly with Q0.
            wmp_tiles = []
            for p3 in range(SLAB // 2):
                t = wmp_pool.tile([128, 768], bf16, tag="wmP")
                wmp_tiles.append(t)
            wm_tiles = [None] * NSLAB

            def fetch_wm_pair(p3):
                nc.sync.dma_start(
                    out=wmp_tiles[p3][:, :],
                    in_=wm_d[:, p3 * 768 : (p3 + 1) * 768],
                )

            def fetch_wm(s):
                t = wm_pool.tile([128, 2304], bf16, tag="wmS")
                wm_tiles[s] = t
                nc.sync.dma_start(
                    out=t[:, :], in_=wm_d[:, s * 2304 : (s + 1) * 2304]
                )

            fetch_wm_pair(0)
            nc.sync.dma_start(out=xw_tiles[0][:, :], in_=xw_d[:, 0 : 8 * 256])
            nc.sync.dma_start(out=xc_all[:, :], in_=xc_d[:, :])
            nc.sync.dma_start(out=we_all[:, :], in_=we_d[:, :])
            fetch_wm_pair(1)
            fetch_wm_pair(2)
            nc.sync.dma_start(
                out=xw_tiles[1][:, :], in_=xw_d[:, 8 * 256 : 16 * 256]
            )
            for s in range(1, NSLAB):
                fetch_wm(s)

            # PE pre-warm: dummy matmuls on zeroed SBUF while the first input
            # DMAs are in flight, so the HAM clock-gate opens (1.2 -> 2.4 GHz)
            # before the real matmul stream begins.
            warm_sb = const_pool.tile([128, 640], bf16, tag="warm")
            nc.vector.memset(warm_sb[:, :], 0.0)
            # The bridge is sized to carry the PE, busy and warm, from the
            # first possible matmul (~8us) to when slabs 0-1 + the first
            # xw tiles have certainly landed (~16us): this part's HAM takes
            # >12us of continuous activity to re-open after any idle gap,
            # so one early famine costs far more than an over-long bridge.
            warm_ps = psum_pool.tile([128, 1024], f32, tag="ps")
            for _ in range(30):
                nc.tensor.matmul(
                    warm_ps[:, 0:512],
                    warm_sb[:, 0:128],
                    warm_sb[:, 128:640],
                    start=True,
                    stop=True,
                )

            def win(w):  # rhs AP for local window index w (full 128 rows)
                ti = 0
                while w >= xw_base[ti] + xw_sizes[ti]:
                    ti += 1
                o = w - xw_base[ti]
                return xw_tiles[ti][:, o * 256 : (o + 1) * 256]

            PAIR_PAD = [3, 3, 2, 2, 2, 1, 1, 1, 1]
            for s in range(NSLAB):
                for q2 in range(SLAB // 2):  # chunk pairs within slab
                    # slab 0 weights live in pair-granular startup tiles
                    if s == 0:
                        wm_sb = wmp_tiles[q2][:, :]
                        wq0 = -q2 * 768  # so q*384 indexes within the pair
                    else:
                        wm_sb = wm_tiles[s][:, :]
                        wq0 = 0
                    pi = s * (SLAB // 2) + q2  # global pair index
                    if pi % 2 == 0:
                        ps = psum_pool.tile([128, 1024], f32)
                    # start=True only on the pair's first MM: it clears the
                    # has_written bits of this pair's whole PSUM bank; every
                    # later MM (start=False) overwrites fresh cells and
                    # accumulates onto written ones, so MM order is free.
                    # window order j, j+2, j+4 so early pairs only touch
                    # early xw tiles (keeps the startup famine-free);
                    # accumulation order within the group is free.
                    for half in range(2):
                        q = q2 * 2 + half
                        j = s * SLAB + q
                        co = (pi % 2) * 512 + half * 256
                        w0 = wq0 + q * 384
                        nc.tensor.matmul(
                            ps[:, co : co + 256],
                            wm_sb[:, w0 + 256 : w0 + 384],
                            win(j),
                            start=(half == 0),
                            stop=False,
                            skip_group_check=True,
                        )
                        nc.tensor.matmul(
                            ps[:, co : co + 256],
                            wm_sb[:, w0 : w0 + 128],
                            win(j + 2),
                            start=False,
                            stop=False,
                            skip_group_check=True,
                        )
                        nc.tensor.matmul(
                            ps[:, co : co + 256],
                            wm_sb[:, w0 + 128 : w0 + 256],
                            win(j + 4),
                            start=False,
                            stop=False,
                            skip_group_check=True,
                        )
                    # packed edge MMs: even chunk in array row group 0,
                    # odd chunk in row group 1 -- they execute concurrently.
                    co = (pi % 2) * 512
                    nc.tensor.matmul(
                        ps[:, co : co + 256],
                        we_sb[0:8, pi * 128 : (pi + 1) * 128],
                        xc_sb[0:8, pi * 256 : (pi + 1) * 256],
                        start=False,
                        stop=False,
                        skip_group_check=True,
                    )
                    nc.tensor.matmul(
                        ps[:, co + 256 : co + 512],
                        weo_sb[0:8, pi * 128 : (pi + 1) * 128],
                        xco_sb[0:8, pi * 256 : (pi + 1) * 256],
                        start=False,
                        stop=True,
                        skip_group_check=True,
                    )
                    # pacing dummies: while the DMA fabric is still ramping,
                    # keep the PE busy between early pairs instead of letting
                    # it stall (a stall > ~3us re-throttles the clock-gate
                    # and costs >12us at half clock).
                    for _ in range(PAIR_PAD[pi] if pi < len(PAIR_PAD) else 0):
                        nc.tensor.matmul(
                            warm_ps[:, 0:512],
                            warm_sb[:, 0:128],
                            warm_sb[:, 128:640],
                            start=True,
                            stop=True,
                        )

                    npair = NSLAB * (SLAB // 2)
                    if s == NSLAB - 1:
                        # tail: per-pair sigmoid+store so the final store is
                        # small and starts right after the last matmuls
                        ot = out_pool.tile([128, 1024], f16)
                        co = (pi % 2) * 512
                        nc.scalar.activation(
                            ot[0:C, 0:512],
                            ps[0:C, co : co + 512],
                            mybir.ActivationFunctionType.Sigmoid,
                            bias=0.0,
                            scale=1.0 / XSCALE,
                        )
                        nc.gpsimd.dma_start(
                            out=yt_d[:, pi * 512 : (pi + 1) * 512],
                            in_=ot[0:C, 0:512],
                        )
                    elif pi % 2 == 1 or pi == npair - 1:
                        width = (pi % 2 + 1) * 512
                        ot = out_pool.tile([128, 1024], f16)
                        nc.scalar.activation(
                            ot[0:C, 0:width],
                            ps[0:C, 0:width],
                            mybir.ActivationFunctionType.Sigmoid,
                            bias=0.0,
                            scale=1.0 / XSCALE,
                        )
                        j0 = (pi // 2) * 4  # first chunk of this store group
                        nc.gpsimd.dma_start(
                            out=yt_d[:, j0 * 256 : j0 * 256 + width],
                            in_=ot[0:C, 0:width],
                        )
    nc.compile()
    return nc


TRACE = False          # set by test harness to capture an NTFF profile
LAST_RESULTS = None    # BassKernelResults of the most recent run
_NC_CACHE = None       # compiled program, reused across calls


def kernel(x, weights, nbr_idx, valid, fault_mask):
    global LAST_RESULTS
    from concourse.bass_utils import run_bass_kernel_spmd

    x = np.asarray(x)
    out_dtype = x.dtype

    WM, WE = _build_blocks(np.asarray(weights), np.asarray(nbr_idx), np.asarray(valid))
    WM = WM.astype(_BF16)
    WE = WE.astype(_BF16)

    # x transposed to (hw, batch), zero-padded, scaled into e3m4 headroom
    xt_pad = np.zeros((PAD + GRID + PAD, BATCH), dtype=_E3M4)
    xt_pad[PAD : PAD + HW] = np.clip(
        np.ascontiguousarray(x.T) * np.float32(XSCALE), -15.5, 15.5
    ).astype(_E3M4)

    k128 = np.arange(128)
    in_maps = []
    for c in range(NCORES):
        j0 = c * CPC
        # main windows: global window t in [j0-2, j0+70), rows PAD + 126*t - 1 + k
        tglob = j0 - 2 + np.arange(NWIN)
        rows = (PAD + C * tglob - 1)[:, None] + k128[None, :]  # (72, 128)
        xw = np.ascontiguousarray(xt_pad[rows].transpose(1, 0, 2))  # (128, 72, 256)

        # merged edge windows: rows 0:4 from [126J+379, +4), 4:8 from [126J-257, +4)
        Jc = j0 + np.arange(CPC)
        k4 = np.arange(4)
        erows_p = (PAD + C * Jc + 379)[:, None] + k4[None, :]  # (66, 4)
        erows_m = (PAD + C * Jc - 257)[:, None] + k4[None, :]  # (66, 4)
        xc = np.concatenate(
            [xt_pad[erows_p], xt_pad[erows_m]], axis=1
        )  # (66, 8, 256)
        xce = xc[0::2].transpose(1, 0, 2)  # (8, 33, 256)
        xco = xc[1::2].transpose(1, 0, 2)

        wm_c = WM[j0 : j0 + CPC].transpose(1, 0, 2)  # (128, 66, 384)
        we_cc = WE[j0 : j0 + CPC]
        wee = we_cc[0::2].transpose(1, 0, 2)  # (8, 33, 128)
        weo = we_cc[1::2].transpose(1, 0, 2)

        in_maps.append(
            {
                "xw": xw.reshape(128, NWIN * 256),
                "wm": np.ascontiguousarray(wm_c).reshape(128, CPC * 384),
                "xc": np.ascontiguousarray(
                    np.concatenate(
                        [xce.reshape(8, -1), xco.reshape(8, -1)], axis=1
                    )
                ),
                "we": np.ascontiguousarray(
                    np.concatenate(
                        [wee.reshape(8, -1), weo.reshape(8, -1)], axis=1
                    )
                ),
            }
        )

    global _NC_CACHE
    if _NC_CACHE is None:
        _NC_CACHE = _build_program()
    nc = _NC_CACHE
    res = run_bass_kernel_spmd(
        nc, in_maps, core_ids=list(range(NCORES)), trace=TRACE
    )
    LAST_RESULTS = res

    # unshard: per-core yt is (126, 66*256) partition-major -> (B, HW)
    yts = [
        r["yt"].reshape(C, CPC, BATCH).transpose(1, 0, 2).reshape(CPC * C, BATCH)
        for r in res.results
    ]
    yt = np.concatenate(yts, axis=0)  # (66528, 256)
    y = np.ascontiguousarray(yt[:HW].T).astype(np.float32, copy=False)
    # faulted units: reference computes sigmoid(where(fault, y, 0)) -> 0.5
    fault = np.asarray(fault_mask).astype(bool)
    y[:, ~fault] = np.float32(0.5)
    return y.astype(out_dtype, copy=False)
